# revision 51
# baseline (speedup 1.0000x reference)
"""Multi-head attention (B=2, S=2048, D=1024, H=16, RoPE, full softmax) on
8 TRN2 NeuronCores.

Sharding: batch x head-group. Core c = 4*b + g handles batch b and heads
[4g, 4g+4). Each core computes q/k/v projections for its 4 heads, RoPE,
scores, softmax, attention, and a partial output projection against its
head-group's w_o columns. The host sums the 4 partial outputs per batch and
adds b_o.

Device layout highlights:
  - x is shipped transposed (xT [1024, 2048] bf16) so the d-contraction sits
    on partitions for both the q/k (w stationary) and v (x stationary)
    projections.
  - q/k weight rows are packed as TE/TO m-tiles (4 heads x 32 even dims,
    then odd dims) so RoPE becomes 4 fused (psum+bias)*table muls plus one
    add/sub per group, all partition-aligned.
  - scores are computed transposed (scoresT[t, s]) with head-PAIR row
    packing: kpair/qpair tiles hold two heads at partitions 0-63 / 64-127,
    so two K=64 matmuls run concurrently on disjoint array row-strips.
  - v carries an extra ones column per head: the attnT matmul's 65th output
    row accumulates the softmax denominator for free.
  - softmax skips max-subtraction (scores are pre-scaled by 1/8 via the RoPE
    tables; |scores| < ~7 so exp is safe in fp32->bf16).
  - each denominator row is spread across 128 partitions via a DRAM bounce
    (DVE reciprocal cost scales with free-dim size, not partitions),
    reciprocal'd, gathered back, and broadcast with stream_shuffle.
  - the output partials are written bf16 (the host accumulates the 4
    partials per batch in fp32), halving the 8MB/core output DMA.
"""

import os
import sys

for _p in ("/opt/trn_rl_repo",):
    if _p not in sys.path and os.path.isdir(_p):
        sys.path.append(_p)

import numpy as np
import ml_dtypes

import concourse.bass as bass
import concourse.mybir as mybir
from concourse.tile import TileContext
from concourse.bass_utils import run_bass_kernel_spmd

F32 = mybir.dt.float32
BF16 = mybir.dt.bfloat16
NPBF16 = ml_dtypes.bfloat16

B, S, D, H = 2, 2048, 1024, 16
HD = D // H          # 64
G = 4                # heads per core
P = 128
NCORES = 8
DC = D // P          # 8 d-chunks
ST = S // P          # 16 t-tiles
SC = S // 512        # 4 s-chunks of 512


# ---------------------------------------------------------------------------
# walrus workaround: this container's walrus rejects >1 sync wait per
# instruction. Hoist extra waits onto NoOps inserted just before the
# instruction on the same engine queue (queues execute in order, so this
# is semantics-preserving).
# ---------------------------------------------------------------------------
def _fix_multiwait(nc, max_waits=1):
    from bass_rust import SyncInfo

    n_split = 0
    for fn in nc.m.functions:
        for bb in fn.blocks:
            insts = bb.instructions
            out = []
            dirty = False
            for ins in insts:
                si = ins.sync_info
                if si is not None and si.on_wait and len(si.on_wait) > max_waits:
                    waits = list(si.on_wait)
                    for i, w in enumerate(waits[:-max_waits]):
                        nop = mybir.InstNoOp(name=f"{ins.name}-mw{i}")
                        nop.engine = ins.engine
                        nop.sync_info = SyncInfo(on_wait=[w], on_update=[])
                        out.append(nop)
                    ins.sync_info = SyncInfo(
                        on_wait=waits[-max_waits:], on_update=list(si.on_update)
                    )
                    dirty = True
                    n_split += 1
                out.append(ins)
            if dirty:
                bb.instructions = out
    return n_split


# ---------------------------------------------------------------------------
# device kernel
# ---------------------------------------------------------------------------
def _build_nc():
    # the exit drain's multi-wait is handled by _fix_multiwait (cheap NOPs)
    nc = bass.Bass()

    xT = nc.declare_dram_parameter("xT", [D, S], BF16, isOutput=False)
    wqkT = nc.declare_dram_parameter("wqkT", [D, 4 * P], BF16, isOutput=False)
    bqk = nc.declare_dram_parameter("bqk", [P, 4], F32, isOutput=False)
    wvT = nc.declare_dram_parameter("wvT", [D, G * HD], BF16, isOutput=False)
    bvb = nc.declare_dram_parameter("bvb", [P, G * HD], F32, isOutput=False)
    cosq = nc.declare_dram_parameter("cosq", [P, S], BF16, isOutput=False)
    sinq = nc.declare_dram_parameter("sinq", [P, S], BF16, isOutput=False)
    cosk = nc.declare_dram_parameter("cosk", [P, S], BF16, isOutput=False)
    sink = nc.declare_dram_parameter("sink", [P, S], BF16, isOutput=False)
    woT = nc.declare_dram_parameter("woT", [G * HD, D], BF16, isOutput=False)
    out = nc.declare_dram_parameter("out", [S, D], BF16, isOutput=True)

    with TileContext(nc) as tc:
        with tc.tile_pool(name="const", bufs=1) as cpool:
            # ---- resident loads -------------------------------------------
            # DMA emission order matters: the SP sequencer dispatches
            # DIRECT2D DMAs serially at ~0.6us each, so order by first-use.
            # K(sc0/1) needs wqk + the first s-half of xT; tables/wv arrive
            # by rope/v time; the xT s-half 2 and w_o trail.
            xT_sb = cpool.tile([P, DC, S], BF16)
            wqk_sb = cpool.tile([P, DC, 4 * P], BF16)
            wv_sb = cpool.tile([P, DC, G * HD], BF16)
            xTr = xT[:].rearrange("(dc p) s -> p dc s", p=P)
            wqkr = wqkT[:].rearrange("(dc p) m -> p dc m", p=P)
            wvr = wvT[:].rearrange("(dc p) m -> p dc m", p=P)
            # k m-tiles (wqk cols 256:512) + the xT s-chunk-0 columns gate
            # the first projection -> dispatch those slices first (the SP
            # sequencer issues DIRECT2D DMAs serially at ~0.6us each).
            for dc in range(DC):
                nc.sync.dma_start(
                    wqk_sb[:, dc, 256:512], wqkr[:, dc, 256:512])
                nc.sync.dma_start(xT_sb[:, dc, 0:512], xTr[:, dc, 0:512])
            bqk_sb = cpool.tile([P, 4], F32)
            nc.sync.dma_start(bqk_sb[:], bqk[:])
            tabs = {}
            for nm, src in (("cosk", cosk), ("sink", sink),
                            ("cosq", cosq), ("sinq", sinq)):
                t = cpool.tile([P, S], BF16, name=f"tab_{nm}")
                nc.sync.dma_start(t[:, 0:512], src[:, 0:512])
                tabs[nm] = t
            for dc in range(DC):
                nc.sync.dma_start(wqk_sb[:, dc, 0:256], wqkr[:, dc, 0:256])
            nc.sync.dma_start(wv_sb[:], wvr[:])
            bvb_sb = cpool.tile([P, G * HD], F32)
            nc.sync.dma_start(bvb_sb[:], bvb[:])
            for dc in range(DC):
                nc.sync.dma_start(xT_sb[:, dc, 512:1024], xTr[:, dc, 512:1024])
            tsrc = {"cosk": cosk, "sink": sink, "cosq": cosq, "sinq": sinq}
            for nm in ("cosk", "sink", "cosq", "sinq"):
                nc.sync.dma_start(tabs[nm][:, 512:2048], tsrc[nm][:, 512:2048])
            for dc in range(DC):
                nc.sync.dma_start(
                    xT_sb[:, dc, 1024:2048], xTr[:, dc, 1024:2048])
            wo_sb = cpool.tile([P, 2, D], BF16)
            nc.sync.dma_start(
                wo_sb[:], woT[:].rearrange("(jc p) d -> p jc d", p=P))

            # pair tiles (2 heads each at partitions 0-63 / 64-127)
            qpair = [cpool.tile([P, S], BF16, name=f"qpair{i}") for i in range(2)]
            kpair = [cpool.tile([P, S], BF16, name=f"kpair{i}") for i in range(2)]
            # v with ones column per head: [p, t_tile, 4*65]
            vext = cpool.tile([P, ST, G * 65], BF16)
            v4 = vext[:].rearrange("p t (h c) -> p t h c", c=65)
            nc.vector.memset(v4[:, :, :, 64:65], 1.0)
            # normalized attention, assembled per pair [128 j, S] for w_o
            attn_n = [cpool.tile([P, S], BF16, name=f"attn{i}") for i in range(2)]
            # per-head raw/normalized attnT staging (base partition 0)
            attn_raw = [cpool.tile([HD + 1, S], BF16, name=f"attnraw{i}")
                        for i in range(4)]
            attn_nh = [cpool.tile([HD, S], BF16, name=f"attnnh{i}")
                       for i in range(4)]

            # ---- helpers --------------------------------------------------
            rtmp_cm = tc.tile_pool(name="rope_t", bufs=3)
            rtmp = rtmp_cm.__enter__()
            if True:
                def rope_group(grp, ps_pair, ssl, use_act=False):
                    psTE, psTO = ps_pair
                    bTE = bqk_sb[:, 2 * grp:2 * grp + 1]
                    bTO = bqk_sb[:, 2 * grp + 1:2 * grp + 2]
                    cosT = tabs["cosq" if grp == 0 else "cosk"]
                    sinT = tabs["sinq" if grp == 0 else "sink"]
                    t1 = rtmp.tile([P, 512], BF16, tag="t1", name="t1")
                    t2 = rtmp.tile([P, 512], BF16, tag="t2", name="t2")
                    t3 = rtmp.tile([P, 512], BF16, tag="t3", name="t3")
                    t4 = rtmp.tile([P, 512], BF16, tag="t4", name="t4")
                    add, mult = mybir.AluOpType.add, mybir.AluOpType.mult
                    ident = mybir.ActivationFunctionType.Identity
                    if use_act:
                        # ACT idle window: evacuate the biased psum through
                        # Scalar, bf16 muls on DVE
                        eTE = rtmp.tile([P, 512], BF16, tag="eTE", name="eTE")
                        eTO = rtmp.tile([P, 512], BF16, tag="eTO", name="eTO")
                        nc.scalar.activation(eTE[:], psTE[:], ident, bias=bTE)
                        nc.scalar.activation(eTO[:], psTO[:], ident, bias=bTO)
                        nc.vector.tensor_mul(t1[:], eTE[:], cosT[:, ssl])
                        nc.vector.tensor_mul(t2[:], eTO[:], sinT[:, ssl])
                        nc.vector.tensor_mul(t3[:], eTE[:], sinT[:, ssl])
                        nc.vector.tensor_mul(t4[:], eTO[:], cosT[:, ssl])
                    else:
                        nc.vector.scalar_tensor_tensor(
                            t1[:], psTE[:], bTE, cosT[:, ssl], op0=add, op1=mult)
                        nc.vector.scalar_tensor_tensor(
                            t2[:], psTO[:], bTO, sinT[:, ssl], op0=add, op1=mult)
                        nc.vector.scalar_tensor_tensor(
                            t3[:], psTE[:], bTE, sinT[:, ssl], op0=add, op1=mult)
                        nc.vector.scalar_tensor_tensor(
                            t4[:], psTO[:], bTO, cosT[:, ssl], op0=add, op1=mult)
                    rotE = rtmp.tile([P, 512], BF16, tag="rotE", name="rotE")
                    rotO = rtmp.tile([P, 512], BF16, tag="rotO", name="rotO")
                    nc.vector.tensor_sub(rotE[:], t1[:], t2[:])
                    nc.vector.tensor_add(rotO[:], t3[:], t4[:])
                    dst = qpair if grp == 0 else kpair
                    for pr in range(2):
                        for half, rot in ((0, rotE), (1, rotO)):
                            for hh in range(2):
                                src_lo = (2 * pr + hh) * 32
                                dst_lo = hh * 64 + half * 32
                                nc.sync.dma_start(
                                    dst[pr][dst_lo:dst_lo + 32, ssl],
                                    rot[src_lo:src_lo + 32, :],
                                )

                def proj_mtile(m, ssl, pool, tag="o"):
                    ps = pool.tile([P, 512], F32, tag=tag, name="psqk")
                    for dc in range(DC):
                        nc.tensor.matmul(
                            ps[:],
                            wqk_sb[:, dc, m * P:(m + 1) * P],
                            xT_sb[:, dc, ssl],
                            start=(dc == 0), stop=(dc == DC - 1),
                        )
                    return ps

            # ---- projections + attention + w_o in one psum scope ----------
            with tc.tile_pool(name="ps_s", bufs=2, space="PSUM") as ps_sp, \
                 tc.tile_pool(name="ps_a", bufs=2, space="PSUM") as ps_ap, \
                 tc.tile_pool(name="ps_o", bufs=2, space="PSUM") as ps_op, \
                 tc.tile_pool(name="p_sb", bufs=8) as ppool, \
                 tc.tile_pool(name="norm", bufs=3) as npool, \
                 tc.tile_pool(name="dscr", bufs=4, space="DRAM") as dpool, \
                 tc.tile_pool(name="o_sb", bufs=3) as opool:
                def attn_groups(pr, sc, tts, psA, psB, mid=None):
                    ssl = slice(sc * 512, (sc + 1) * 512)
                    for tt in tts:
                        if mid is not None and tt == 8:
                            mid()
                        pss = ps_sp.tile([P, 1024], F32, tag="sc", name="pss")
                        nc.tensor.matmul(
                            pss[:, 0:512],
                            kpair[pr][0:64, tt * P:(tt + 1) * P],
                            qpair[pr][0:64, ssl],
                            start=True, stop=True)
                        nc.tensor.matmul(
                            pss[:, 512:1024],
                            kpair[pr][64:128, tt * P:(tt + 1) * P],
                            qpair[pr][64:128, ssl],
                            start=True, stop=True)
                        p_sb = ppool.tile([P, 1024], BF16, tag="p", name="p_sb")
                        nc.scalar.activation(
                            p_sb[:], pss[:], mybir.ActivationFunctionType.Exp)
                        hA, hB = 2 * pr, 2 * pr + 1
                        nc.tensor.matmul(
                            psA[:],
                            vext[:, tt, hA * 65:hA * 65 + 65],
                            p_sb[:, 0:512],
                            start=(tt == 0), stop=(tt == ST - 1))
                        nc.tensor.matmul(
                            psB[:],
                            vext[:, tt, hB * 65:hB * 65 + 65],
                            p_sb[:, 512:1024],
                            start=(tt == 0), stop=(tt == ST - 1))

                def attn_norm(pr, sc, psA, psB):
                    # normalize straight out of PSUM: spread the 65th row
                    # across partitions by DMA, recip (free-size-bound),
                    # gather back, broadcast via stream_shuffle
                    ssl = slice(sc * 512, (sc + 1) * 512)
                    for hh, psX in ((0, psA), (1, psB)):
                        h = 2 * pr + hh
                        # single 65-row evacuation: rows 0-63 = attnT, row
                        # 64 = denominator (bf16 is plenty for a softmax sum)
                        nc.vector.tensor_copy(attn_raw[h][:, ssl], psX[:, :])
                        dr1 = dpool.tile([512], BF16, tag="dr1", name="dr1")
                        nc.sync.dma_start(dr1[:], attn_raw[h][64:65, ssl])
                        dsc = npool.tile([P, 4], BF16, tag="dsc", name="dsc")
                        nc.sync.dma_start(
                            dsc[:], dr1[:].rearrange("(p c) -> p c", p=P))
                        drc = npool.tile([P, 4], F32, tag="drc", name="drc")
                        nc.vector.reciprocal(drc[:], dsc[:])
                        dr2 = dpool.tile([512], F32, tag="dr2", name="dr2")
                        nc.sync.dma_start(
                            dr2[:].rearrange("(p c) -> p c", p=P), drc[:])
                        dg = npool.tile([32, 512], F32, tag="dg", name="dg")
                        nc.sync.dma_start(dg[0:1, :], dr2[:])
                        bc = npool.tile([HD, 512], F32, tag="bc", name="bc")
                        nc.vector.stream_shuffle(
                            bc[0:32, :], dg[:, :], mask=[0] * 32)
                        nc.vector.stream_shuffle(
                            bc[32:64, :], dg[:, :], mask=[0] * 32)
                        nc.vector.tensor_mul(
                            attn_nh[h][:, ssl], attn_raw[h][0:64, ssl], bc[:])
                        nc.sync.dma_start(
                            attn_n[pr][hh * 64:(hh + 1) * 64, ssl],
                            attn_nh[h][:, ssl])

                # projections: k, q, v per s-chunk through the tag-"o"
                # psum ring (shared with the late w_o phase). Attention is
                # emitted after (program order = semantic order), but its
                # psum pools are untouched here so the scheduler overlaps
                # its execution with the tail of these projections.
                # preload the exp ACT table while input DMAs land so the
                # first real exp doesn't pay the ~2.7us table-load
                warm = rtmp.tile([1, 16], F32, tag="t1", name="warm")
                nc.vector.memset(warm[:], 0.0)
                warm2 = rtmp.tile([1, 16], BF16, tag="t2", name="warm2")
                nc.scalar.activation(warm2[:], warm[:],
                                     mybir.ActivationFunctionType.Exp)
                psA0 = ps_ap.tile([65, 512], F32, tag="at", name="psA0")
                psB0 = ps_ap.tile([65, 512], F32, tag="at", name="psB0")
                for sc in range(SC):
                    ssl = slice(sc * 512, (sc + 1) * 512)
                    rope_group(
                        1,
                        [proj_mtile(2, ssl, ps_op),
                         proj_mtile(3, ssl, ps_op)],
                        ssl, use_act=(sc == 0))
                    if sc == 0:
                        rope_group(
                            0,
                            [proj_mtile(0, ssl, ps_op),
                             proj_mtile(1, ssl, ps_op)],
                            ssl)
                    for tl in range(4):
                        tt = sc * 4 + tl
                        psv = ps_op.tile([P, G * HD], F32, tag="o", name="psv")
                        for dc in range(DC):
                            nc.tensor.matmul(
                                psv[:],
                                xT_sb[:, dc, tt * P:(tt + 1) * P],
                                wv_sb[:, dc, :],
                                start=(dc == 0), stop=(dc == DC - 1),
                            )
                        for h in range(G):
                            nc.vector.tensor_add(
                                vext[:, tt, h * 65:h * 65 + 64],
                                psv[:, h * HD:(h + 1) * HD],
                                bvb_sb[:, h * HD:(h + 1) * HD],
                            )
                    # (pair 0, s-chunk 0) attention groups for THIS step's
                    # t-tiles: kpair/vext for them were just produced, and
                    # qpair(sc0) exists from step 0, so these run immediately
                    # and start the exp stream during production
                    attn_groups(0, 0, range(sc * 4, sc * 4 + 4), psA0, psB0)
                def wo_block(sts):
                    # one [128,1024] staging tile and ONE out-DMA per
                    # s-tile: halves the tail's serialized DMA dispatches.
                    # Evacuations alternate DVE / Scalar-Identity (Scalar
                    # is idle by then and has the fast PSUM port), halving
                    # the serial copy chain.
                    ident = mybir.ActivationFunctionType.Identity
                    for st in sts:
                        osb = opool.tile([P, 1024], BF16, tag="ot", name="osb")
                        for half in range(2):
                            pso = ps_op.tile([P, 512], F32, tag="o", name="pso")
                            for jc in range(2):
                                nc.tensor.matmul(
                                    pso[:],
                                    attn_n[jc][:, st * P:(st + 1) * P],
                                    wo_sb[:, jc, half * 512:(half + 1) * 512],
                                    start=(jc == 0), stop=(jc == 1))
                            dstv = osb[:, half * 512:(half + 1) * 512]
                            if half == 0:
                                nc.vector.tensor_copy(dstv, pso[:])
                            else:
                                nc.scalar.activation(dstv, pso[:], ident)
                        nc.sync.dma_start(
                            out[st * P:(st + 1) * P, :], osb[:])

                def q_proj(sc):
                    ssl = slice(sc * 512, (sc + 1) * 512)
                    rope_group(
                        0,
                        [proj_mtile(0, ssl, ps_op),
                         proj_mtile(1, ssl, ps_op)],
                        ssl)

                attn_norm(0, 0, psA0, psB0)
                for pr in range(2):
                    for sc in range(SC):
                        if pr == 0 and sc == 0:
                            continue
                        if pr == 0 and sc == 1:
                            q_proj(1)
                        # q for the NEXT pr0 combo is emitted mid-combo:
                        # its ~7us proj+rope+DMA chain completes in this
                        # combo's PE slack instead of stalling the exp
                        # stream at the combo boundary
                        mid = None
                        if pr == 0 and sc < SC - 1:
                            mid = (lambda s=sc + 1: q_proj(s))
                        psA = ps_ap.tile([65, 512], F32, tag="at", name="psA")
                        psB = ps_ap.tile([65, 512], F32, tag="at", name="psB")
                        attn_groups(pr, sc, range(ST), psA, psB, mid=mid)
                        attn_norm(pr, sc, psA, psB)
                wo_block(range(ST))

            rtmp_cm.__exit__(None, None, None)

    _fix_multiwait(nc)
    return nc


_NC_CACHE = None


def _get_nc():
    global _NC_CACHE
    if _NC_CACHE is None:
        _NC_CACHE = _build_nc()
    return _NC_CACHE


# ---------------------------------------------------------------------------
# host-side sharding
# ---------------------------------------------------------------------------
def _deint(rows):
    """rows [64, ...] -> [even dims (32); odd dims (32)]"""
    return np.concatenate([rows[0::2], rows[1::2]], axis=0)


def _shard_inputs(input, rotations, w_qkv, b_qkv, w_o, b_o):
    x = np.asarray(input, np.float32)
    rot = np.asarray(rotations, np.float32)
    w_qkv = np.asarray(w_qkv, np.float32)
    b_qkv = np.asarray(b_qkv, np.float32)
    w_o = np.asarray(w_o, np.float32)

    cos = rot[:, :, 0].T.copy()   # [32, S]
    sin = rot[:, :, 1].T.copy()
    cos4 = np.tile(cos, (4, 1))   # [128, S]
    sin4 = np.tile(sin, (4, 1))
    cosq = (cos4 / 8.0).astype(NPBF16)
    sinq = (sin4 / 8.0).astype(NPBF16)
    cosk = cos4.astype(NPBF16)
    sink = sin4.astype(NPBF16)

    in_maps = []
    for c in range(NCORES):
        b, g = divmod(c, 4)
        heads = [4 * g + i for i in range(G)]
        xT = np.ascontiguousarray(x[b].T).astype(NPBF16)          # [D, S]

        # q/k m-tiles: TE then TO, 4 heads x 32 rows each, for q then k
        q_te, q_to, k_te, k_to, bq_te, bq_to, bk_te, bk_to = \
            [], [], [], [], [], [], [], []
        for h in heads:
            qw = _deint(w_qkv[h * HD:(h + 1) * HD])
            kw = _deint(w_qkv[D + h * HD:D + (h + 1) * HD])
            qb = _deint(b_qkv[h * HD:(h + 1) * HD])
            kb = _deint(b_qkv[D + h * HD:D + (h + 1) * HD])
            q_te.append(qw[:32]); q_to.append(qw[32:])
            k_te.append(kw[:32]); k_to.append(kw[32:])
            bq_te.append(qb[:32]); bq_to.append(qb[32:])
            bk_te.append(kb[:32]); bk_to.append(kb[32:])
        wqk = np.concatenate(
            [np.concatenate(blk, axis=0) for blk in (q_te, q_to, k_te, k_to)],
            axis=0)                                                # [512, D]
        wqkT = np.ascontiguousarray(wqk.T).astype(NPBF16)          # [D, 512]
        bqk = np.stack(
            [np.concatenate(blk) for blk in (bq_te, bq_to, bk_te, bk_to)],
            axis=1).astype(np.float32)                             # [128, 4]

        wv = np.concatenate(
            [w_qkv[2 * D + h * HD:2 * D + (h + 1) * HD] for h in heads], axis=0)
        wvT = np.ascontiguousarray(wv.T).astype(NPBF16)            # [D, 256]
        bv = np.concatenate(
            [b_qkv[2 * D + h * HD:2 * D + (h + 1) * HD] for h in heads])
        bvb = np.tile(bv[None, :], (P, 1)).astype(np.float32)      # [128, 256]

        wo = w_o[:, g * G * HD:(g + 1) * G * HD]                   # [D, 256]
        woT = np.ascontiguousarray(wo.T).astype(NPBF16)            # [256, D]

        in_maps.append({
            "xT": xT, "wqkT": wqkT, "bqk": bqk, "wvT": wvT, "bvb": bvb,
            "cosq": cosq, "sinq": sinq, "cosk": cosk, "sink": sink,
            "woT": woT,
        })
    return in_maps


def _run(inputs, trace=False):
    nc = _get_nc()
    in_maps = _shard_inputs(**inputs)
    res = run_bass_kernel_spmd(
        nc, in_maps, core_ids=list(range(NCORES)), trace=trace)
    b_o = np.asarray(inputs["b_o"], np.float32)
    out = np.zeros((B, S, D), np.float32)
    for c in range(NCORES):
        out[c // 4] += res.results[c]["out"].astype(np.float32)
    out += b_o[None, None, :]
    return out, res


def kernel(**inputs):
    out, _ = _run(inputs, trace=False)
    return out


# revision 56
# speedup vs baseline: 1.0139x; 1.0139x over previous
"""Multi-head attention (B=2, S=2048, D=1024, H=16, RoPE, full softmax) on
8 TRN2 NeuronCores.

Sharding: batch x head-group. Core c = 4*b + g handles batch b and heads
[4g, 4g+4). Each core computes q/k/v projections for its 4 heads, RoPE,
scores, softmax, attention, and a partial output projection against its
head-group's w_o columns. The host sums the 4 partial outputs per batch and
adds b_o.

Device layout highlights:
  - x is shipped transposed (xT [1024, 2048] bf16) so the d-contraction sits
    on partitions for both the q/k (w stationary) and v (x stationary)
    projections.
  - q/k weight rows are packed as TE/TO m-tiles (4 heads x 32 even dims,
    then odd dims) so RoPE becomes 4 fused (psum+bias)*table muls plus one
    add/sub per group, all partition-aligned.
  - scores are computed transposed (scoresT[t, s]) with head-PAIR row
    packing: kpair/qpair tiles hold two heads at partitions 0-63 / 64-127,
    so two K=64 matmuls run concurrently on disjoint array row-strips.
  - v carries an extra ones column per head: the attnT matmul's 65th output
    row accumulates the softmax denominator for free.
  - softmax skips max-subtraction (scores are pre-scaled by 1/8 via the RoPE
    tables; |scores| < ~7 so exp is safe in fp32->bf16).
  - each denominator row is spread across 128 partitions via a DRAM bounce
    (DVE reciprocal cost scales with free-dim size, not partitions),
    reciprocal'd, gathered back, and broadcast with stream_shuffle.
  - the output partials are written bf16 (the host accumulates the 4
    partials per batch in fp32), halving the 8MB/core output DMA.
"""

import os
import sys

for _p in ("/opt/trn_rl_repo",):
    if _p not in sys.path and os.path.isdir(_p):
        sys.path.append(_p)

import numpy as np
import ml_dtypes

import concourse.bass as bass
import concourse.mybir as mybir
from concourse.tile import TileContext
from concourse.bass_utils import run_bass_kernel_spmd

F32 = mybir.dt.float32
BF16 = mybir.dt.bfloat16
NPBF16 = ml_dtypes.bfloat16

B, S, D, H = 2, 2048, 1024, 16
HD = D // H          # 64
G = 4                # heads per core
P = 128
NCORES = 8
DC = D // P          # 8 d-chunks
ST = S // P          # 16 t-tiles
SC = S // 512        # 4 s-chunks of 512


# ---------------------------------------------------------------------------
# walrus workaround: this container's walrus rejects >1 sync wait per
# instruction. Hoist extra waits onto NoOps inserted just before the
# instruction on the same engine queue (queues execute in order, so this
# is semantics-preserving).
# ---------------------------------------------------------------------------
def _fix_multiwait(nc, max_waits=1):
    from bass_rust import SyncInfo

    n_split = 0
    for fn in nc.m.functions:
        for bb in fn.blocks:
            insts = bb.instructions
            out = []
            dirty = False
            for ins in insts:
                si = ins.sync_info
                if si is not None and si.on_wait and len(si.on_wait) > max_waits:
                    waits = list(si.on_wait)
                    for i, w in enumerate(waits[:-max_waits]):
                        nop = mybir.InstNoOp(name=f"{ins.name}-mw{i}")
                        nop.engine = ins.engine
                        nop.sync_info = SyncInfo(on_wait=[w], on_update=[])
                        out.append(nop)
                    ins.sync_info = SyncInfo(
                        on_wait=waits[-max_waits:], on_update=list(si.on_update)
                    )
                    dirty = True
                    n_split += 1
                out.append(ins)
            if dirty:
                bb.instructions = out
    return n_split


# ---------------------------------------------------------------------------
# device kernel
# ---------------------------------------------------------------------------
def _build_nc():
    # the exit drain's multi-wait is handled by _fix_multiwait (cheap NOPs)
    nc = bass.Bass()

    xT = nc.declare_dram_parameter("xT", [D, S], BF16, isOutput=False)
    wqkT = nc.declare_dram_parameter("wqkT", [D, 4 * P], BF16, isOutput=False)
    bqk = nc.declare_dram_parameter("bqk", [P, 4], F32, isOutput=False)
    wvT = nc.declare_dram_parameter("wvT", [D, G * HD], BF16, isOutput=False)
    bvb = nc.declare_dram_parameter("bvb", [P, G * HD], F32, isOutput=False)
    cosq = nc.declare_dram_parameter("cosq", [P, S], BF16, isOutput=False)
    sinq = nc.declare_dram_parameter("sinq", [P, S], BF16, isOutput=False)
    cosk = nc.declare_dram_parameter("cosk", [P, S], BF16, isOutput=False)
    sink = nc.declare_dram_parameter("sink", [P, S], BF16, isOutput=False)
    woT = nc.declare_dram_parameter("woT", [G * HD, D], BF16, isOutput=False)
    out = nc.declare_dram_parameter("out", [S, D], BF16, isOutput=True)

    with TileContext(nc) as tc:
        with tc.tile_pool(name="const", bufs=1) as cpool:
            # ---- resident loads -------------------------------------------
            # DMA emission order matters: the SP sequencer dispatches
            # DIRECT2D DMAs serially at ~0.6us each, so order by first-use.
            # K(sc0/1) needs wqk + the first s-half of xT; tables/wv arrive
            # by rope/v time; the xT s-half 2 and w_o trail.
            xT_sb = cpool.tile([P, DC, S], BF16)
            wqk_sb = cpool.tile([P, DC, 4 * P], BF16)
            wv_sb = cpool.tile([P, DC, G * HD], BF16)
            xTr = xT[:].rearrange("(dc p) s -> p dc s", p=P)
            wqkr = wqkT[:].rearrange("(dc p) m -> p dc m", p=P)
            wvr = wvT[:].rearrange("(dc p) m -> p dc m", p=P)
            # k m-tiles (wqk cols 256:512) + the xT s-chunk-0 columns gate
            # the first projection -> dispatch those slices first (the SP
            # sequencer issues DIRECT2D DMAs serially at ~0.6us each).
            for dc in range(DC):
                nc.sync.dma_start(
                    wqk_sb[:, dc, 256:512], wqkr[:, dc, 256:512])
                nc.sync.dma_start(xT_sb[:, dc, 0:512], xTr[:, dc, 0:512])
            bqk_sb = cpool.tile([P, 4], F32)
            nc.sync.dma_start(bqk_sb[:], bqk[:])
            tabs = {}
            for nm, src in (("cosk", cosk), ("sink", sink),
                            ("cosq", cosq), ("sinq", sinq)):
                t = cpool.tile([P, S], BF16, name=f"tab_{nm}")
                nc.sync.dma_start(t[:, 0:512], src[:, 0:512])
                tabs[nm] = t
            for dc in range(DC):
                nc.sync.dma_start(wqk_sb[:, dc, 0:256], wqkr[:, dc, 0:256])
            nc.sync.dma_start(wv_sb[:], wvr[:])
            bvb_sb = cpool.tile([P, G * HD], F32)
            nc.sync.dma_start(bvb_sb[:], bvb[:])
            for dc in range(DC):
                nc.sync.dma_start(xT_sb[:, dc, 512:1024], xTr[:, dc, 512:1024])
            tsrc = {"cosk": cosk, "sink": sink, "cosq": cosq, "sinq": sinq}
            for nm in ("cosk", "sink", "cosq", "sinq"):
                nc.sync.dma_start(tabs[nm][:, 512:2048], tsrc[nm][:, 512:2048])
            for dc in range(DC):
                nc.sync.dma_start(
                    xT_sb[:, dc, 1024:2048], xTr[:, dc, 1024:2048])
            wo_sb = cpool.tile([P, 2, D], BF16)
            nc.sync.dma_start(
                wo_sb[:], woT[:].rearrange("(jc p) d -> p jc d", p=P))

            # pair tiles (2 heads each at partitions 0-63 / 64-127)
            qpair = [cpool.tile([P, S], BF16, name=f"qpair{i}") for i in range(2)]
            kpair = [cpool.tile([P, S], BF16, name=f"kpair{i}") for i in range(2)]
            # v with ones column per head: [p, t_tile, 4*65]
            vext = cpool.tile([P, ST, G * 65], BF16)
            v4 = vext[:].rearrange("p t (h c) -> p t h c", c=65)
            nc.vector.memset(v4[:, :, :, 64:65], 1.0)
            # normalized attention, assembled per pair [128 j, S] for w_o
            attn_n = [cpool.tile([P, S], BF16, name=f"attn{i}") for i in range(2)]
            # per-head raw/normalized attnT staging (base partition 0)
            attn_raw = [cpool.tile([HD + 1, S], BF16, name=f"attnraw{i}")
                        for i in range(4)]
            attn_nh = [cpool.tile([HD, S], BF16, name=f"attnnh{i}")
                       for i in range(4)]

            # ---- helpers --------------------------------------------------
            rtmp_cm = tc.tile_pool(name="rope_t", bufs=3)
            rtmp = rtmp_cm.__enter__()
            if True:
                def rope_group(grp, ps_pair, ssl, use_act=False):
                    psTE, psTO = ps_pair
                    bTE = bqk_sb[:, 2 * grp:2 * grp + 1]
                    bTO = bqk_sb[:, 2 * grp + 1:2 * grp + 2]
                    cosT = tabs["cosq" if grp == 0 else "cosk"]
                    sinT = tabs["sinq" if grp == 0 else "sink"]
                    t1 = rtmp.tile([P, 512], BF16, tag="t1", name="t1")
                    t2 = rtmp.tile([P, 512], BF16, tag="t2", name="t2")
                    t3 = rtmp.tile([P, 512], BF16, tag="t3", name="t3")
                    t4 = rtmp.tile([P, 512], BF16, tag="t4", name="t4")
                    add, mult = mybir.AluOpType.add, mybir.AluOpType.mult
                    ident = mybir.ActivationFunctionType.Identity
                    if use_act:
                        # ACT idle window: evacuate the biased psum through
                        # Scalar, bf16 muls on DVE
                        eTE = rtmp.tile([P, 512], BF16, tag="eTE", name="eTE")
                        eTO = rtmp.tile([P, 512], BF16, tag="eTO", name="eTO")
                        nc.scalar.activation(eTE[:], psTE[:], ident, bias=bTE)
                        nc.scalar.activation(eTO[:], psTO[:], ident, bias=bTO)
                        nc.vector.tensor_mul(t1[:], eTE[:], cosT[:, ssl])
                        nc.vector.tensor_mul(t2[:], eTO[:], sinT[:, ssl])
                        nc.vector.tensor_mul(t3[:], eTE[:], sinT[:, ssl])
                        nc.vector.tensor_mul(t4[:], eTO[:], cosT[:, ssl])
                    else:
                        nc.vector.scalar_tensor_tensor(
                            t1[:], psTE[:], bTE, cosT[:, ssl], op0=add, op1=mult)
                        nc.vector.scalar_tensor_tensor(
                            t2[:], psTO[:], bTO, sinT[:, ssl], op0=add, op1=mult)
                        nc.vector.scalar_tensor_tensor(
                            t3[:], psTE[:], bTE, sinT[:, ssl], op0=add, op1=mult)
                        nc.vector.scalar_tensor_tensor(
                            t4[:], psTO[:], bTO, cosT[:, ssl], op0=add, op1=mult)
                    rotE = rtmp.tile([P, 512], BF16, tag="rotE", name="rotE")
                    rotO = rtmp.tile([P, 512], BF16, tag="rotO", name="rotO")
                    nc.vector.tensor_sub(rotE[:], t1[:], t2[:])
                    nc.vector.tensor_add(rotO[:], t3[:], t4[:])
                    dst = qpair if grp == 0 else kpair
                    for pr in range(2):
                        for half, rot in ((0, rotE), (1, rotO)):
                            for hh in range(2):
                                src_lo = (2 * pr + hh) * 32
                                dst_lo = hh * 64 + half * 32
                                # GpSimd queue: these must not wait behind
                                # the resident-load dispatch backlog on the
                                # SP sequencer (rope output gates the first
                                # scores+exp of each s-chunk)
                                nc.gpsimd.dma_start(
                                    dst[pr][dst_lo:dst_lo + 32, ssl],
                                    rot[src_lo:src_lo + 32, :],
                                )

                def proj_mtile(m, ssl, pool, tag="o"):
                    ps = pool.tile([P, 512], F32, tag=tag, name="psqk")
                    for dc in range(DC):
                        nc.tensor.matmul(
                            ps[:],
                            wqk_sb[:, dc, m * P:(m + 1) * P],
                            xT_sb[:, dc, ssl],
                            start=(dc == 0), stop=(dc == DC - 1),
                        )
                    return ps

            # ---- projections + attention + w_o in one psum scope ----------
            with tc.tile_pool(name="ps_s", bufs=2, space="PSUM") as ps_sp, \
                 tc.tile_pool(name="ps_a", bufs=2, space="PSUM") as ps_ap, \
                 tc.tile_pool(name="ps_o", bufs=2, space="PSUM") as ps_op, \
                 tc.tile_pool(name="p_sb", bufs=8) as ppool, \
                 tc.tile_pool(name="norm", bufs=3) as npool, \
                 tc.tile_pool(name="dscr", bufs=4, space="DRAM") as dpool, \
                 tc.tile_pool(name="o_sb", bufs=3) as opool:
                def attn_groups(pr, sc, tts, psA, psB):
                    ssl = slice(sc * 512, (sc + 1) * 512)
                    for tt in tts:
                        pss = ps_sp.tile([P, 1024], F32, tag="sc", name="pss")
                        nc.tensor.matmul(
                            pss[:, 0:512],
                            kpair[pr][0:64, tt * P:(tt + 1) * P],
                            qpair[pr][0:64, ssl],
                            start=True, stop=True)
                        nc.tensor.matmul(
                            pss[:, 512:1024],
                            kpair[pr][64:128, tt * P:(tt + 1) * P],
                            qpair[pr][64:128, ssl],
                            start=True, stop=True)
                        p_sb = ppool.tile([P, 1024], BF16, tag="p", name="p_sb")
                        nc.scalar.activation(
                            p_sb[:], pss[:], mybir.ActivationFunctionType.Exp)
                        hA, hB = 2 * pr, 2 * pr + 1
                        nc.tensor.matmul(
                            psA[:],
                            vext[:, tt, hA * 65:hA * 65 + 65],
                            p_sb[:, 0:512],
                            start=(tt == 0), stop=(tt == ST - 1))
                        nc.tensor.matmul(
                            psB[:],
                            vext[:, tt, hB * 65:hB * 65 + 65],
                            p_sb[:, 512:1024],
                            start=(tt == 0), stop=(tt == ST - 1))

                def attn_norm(pr, sc, psA, psB):
                    # normalize straight out of PSUM: spread the 65th row
                    # across partitions by DMA, recip (free-size-bound),
                    # gather back, broadcast via stream_shuffle
                    ssl = slice(sc * 512, (sc + 1) * 512)
                    for hh, psX in ((0, psA), (1, psB)):
                        h = 2 * pr + hh
                        # single 65-row evacuation: rows 0-63 = attnT, row
                        # 64 = denominator (bf16 is plenty for a softmax sum)
                        nc.vector.tensor_copy(attn_raw[h][:, ssl], psX[:, :])
                        dr1 = dpool.tile([512], BF16, tag="dr1", name="dr1")
                        nc.sync.dma_start(dr1[:], attn_raw[h][64:65, ssl])
                        dsc = npool.tile([P, 4], BF16, tag="dsc", name="dsc")
                        nc.sync.dma_start(
                            dsc[:], dr1[:].rearrange("(p c) -> p c", p=P))
                        drc = npool.tile([P, 4], F32, tag="drc", name="drc")
                        nc.vector.reciprocal(drc[:], dsc[:])
                        dr2 = dpool.tile([512], F32, tag="dr2", name="dr2")
                        nc.sync.dma_start(
                            dr2[:].rearrange("(p c) -> p c", p=P), drc[:])
                        dg = npool.tile([32, 512], F32, tag="dg", name="dg")
                        nc.sync.dma_start(dg[0:1, :], dr2[:])
                        bc = npool.tile([HD, 512], F32, tag="bc", name="bc")
                        nc.vector.stream_shuffle(
                            bc[0:32, :], dg[:, :], mask=[0] * 32)
                        nc.vector.stream_shuffle(
                            bc[32:64, :], dg[:, :], mask=[0] * 32)
                        nc.vector.tensor_mul(
                            attn_nh[h][:, ssl], attn_raw[h][0:64, ssl], bc[:])
                        nc.sync.dma_start(
                            attn_n[pr][hh * 64:(hh + 1) * 64, ssl],
                            attn_nh[h][:, ssl])

                # projections: k, q, v per s-chunk through the tag-"o"
                # psum ring (shared with the late w_o phase). Attention is
                # emitted after (program order = semantic order), but its
                # psum pools are untouched here so the scheduler overlaps
                # its execution with the tail of these projections.
                # preload the exp ACT table while input DMAs land so the
                # first real exp doesn't pay the ~2.7us table-load
                warm = rtmp.tile([1, 16], F32, tag="t1", name="warm")
                nc.vector.memset(warm[:], 0.0)
                warm2 = rtmp.tile([1, 16], BF16, tag="t2", name="warm2")
                nc.scalar.activation(warm2[:], warm[:],
                                     mybir.ActivationFunctionType.Exp)
                psA0 = ps_ap.tile([65, 512], F32, tag="at", name="psA0")
                psB0 = ps_ap.tile([65, 512], F32, tag="at", name="psB0")
                for sc in range(SC):
                    ssl = slice(sc * 512, (sc + 1) * 512)
                    rope_group(
                        1,
                        [proj_mtile(2, ssl, ps_op),
                         proj_mtile(3, ssl, ps_op)],
                        ssl, use_act=(sc == 0))
                    if sc == 0:
                        rope_group(
                            0,
                            [proj_mtile(0, ssl, ps_op),
                             proj_mtile(1, ssl, ps_op)],
                            ssl)
                    for tl in range(4):
                        tt = sc * 4 + tl
                        psv = ps_op.tile([P, G * HD], F32, tag="o", name="psv")
                        for dc in range(DC):
                            nc.tensor.matmul(
                                psv[:],
                                xT_sb[:, dc, tt * P:(tt + 1) * P],
                                wv_sb[:, dc, :],
                                start=(dc == 0), stop=(dc == DC - 1),
                            )
                        for h in range(G):
                            nc.vector.tensor_add(
                                vext[:, tt, h * 65:h * 65 + 64],
                                psv[:, h * HD:(h + 1) * HD],
                                bvb_sb[:, h * HD:(h + 1) * HD],
                            )
                    # (pair 0, s-chunk 0) attention groups for THIS step's
                    # t-tiles: kpair/vext for them were just produced, and
                    # qpair(sc0) exists from step 0, so these run immediately
                    # and start the exp stream during production
                    attn_groups(0, 0, range(sc * 4, sc * 4 + 4), psA0, psB0)
                def wo_block(sts):
                    # one [128,1024] staging tile and ONE out-DMA per
                    # s-tile: halves the tail's serialized DMA dispatches.
                    for st in sts:
                        osb = opool.tile([P, 1024], BF16, tag="ot", name="osb")
                        for half in range(2):
                            pso = ps_op.tile([P, 512], F32, tag="o", name="pso")
                            for jc in range(2):
                                nc.tensor.matmul(
                                    pso[:],
                                    attn_n[jc][:, st * P:(st + 1) * P],
                                    wo_sb[:, jc, half * 512:(half + 1) * 512],
                                    start=(jc == 0), stop=(jc == 1))
                            nc.vector.tensor_copy(
                                osb[:, half * 512:(half + 1) * 512], pso[:])
                        nc.sync.dma_start(
                            out[st * P:(st + 1) * P, :], osb[:])

                attn_norm(0, 0, psA0, psB0)
                for pr in range(2):
                    for sc in range(SC):
                        if pr == 0 and sc == 0:
                            continue
                        if pr == 0:
                            ssl = slice(sc * 512, (sc + 1) * 512)
                            rope_group(
                                0,
                                [proj_mtile(0, ssl, ps_op),
                                 proj_mtile(1, ssl, ps_op)],
                                ssl)
                        psA = ps_ap.tile([65, 512], F32, tag="at", name="psA")
                        psB = ps_ap.tile([65, 512], F32, tag="at", name="psB")
                        attn_groups(pr, sc, range(ST), psA, psB)
                        attn_norm(pr, sc, psA, psB)
                wo_block(range(ST))

            rtmp_cm.__exit__(None, None, None)

    _fix_multiwait(nc)
    return nc


_NC_CACHE = None


def _get_nc():
    global _NC_CACHE
    if _NC_CACHE is None:
        _NC_CACHE = _build_nc()
    return _NC_CACHE


# ---------------------------------------------------------------------------
# host-side sharding
# ---------------------------------------------------------------------------
def _deint(rows):
    """rows [64, ...] -> [even dims (32); odd dims (32)]"""
    return np.concatenate([rows[0::2], rows[1::2]], axis=0)


def _shard_inputs(input, rotations, w_qkv, b_qkv, w_o, b_o):
    x = np.asarray(input, np.float32)
    rot = np.asarray(rotations, np.float32)
    w_qkv = np.asarray(w_qkv, np.float32)
    b_qkv = np.asarray(b_qkv, np.float32)
    w_o = np.asarray(w_o, np.float32)

    cos = rot[:, :, 0].T.copy()   # [32, S]
    sin = rot[:, :, 1].T.copy()
    cos4 = np.tile(cos, (4, 1))   # [128, S]
    sin4 = np.tile(sin, (4, 1))
    cosq = (cos4 / 8.0).astype(NPBF16)
    sinq = (sin4 / 8.0).astype(NPBF16)
    cosk = cos4.astype(NPBF16)
    sink = sin4.astype(NPBF16)

    in_maps = []
    for c in range(NCORES):
        b, g = divmod(c, 4)
        heads = [4 * g + i for i in range(G)]
        xT = np.ascontiguousarray(x[b].T).astype(NPBF16)          # [D, S]

        # q/k m-tiles: TE then TO, 4 heads x 32 rows each, for q then k
        q_te, q_to, k_te, k_to, bq_te, bq_to, bk_te, bk_to = \
            [], [], [], [], [], [], [], []
        for h in heads:
            qw = _deint(w_qkv[h * HD:(h + 1) * HD])
            kw = _deint(w_qkv[D + h * HD:D + (h + 1) * HD])
            qb = _deint(b_qkv[h * HD:(h + 1) * HD])
            kb = _deint(b_qkv[D + h * HD:D + (h + 1) * HD])
            q_te.append(qw[:32]); q_to.append(qw[32:])
            k_te.append(kw[:32]); k_to.append(kw[32:])
            bq_te.append(qb[:32]); bq_to.append(qb[32:])
            bk_te.append(kb[:32]); bk_to.append(kb[32:])
        wqk = np.concatenate(
            [np.concatenate(blk, axis=0) for blk in (q_te, q_to, k_te, k_to)],
            axis=0)                                                # [512, D]
        wqkT = np.ascontiguousarray(wqk.T).astype(NPBF16)          # [D, 512]
        bqk = np.stack(
            [np.concatenate(blk) for blk in (bq_te, bq_to, bk_te, bk_to)],
            axis=1).astype(np.float32)                             # [128, 4]

        wv = np.concatenate(
            [w_qkv[2 * D + h * HD:2 * D + (h + 1) * HD] for h in heads], axis=0)
        wvT = np.ascontiguousarray(wv.T).astype(NPBF16)            # [D, 256]
        bv = np.concatenate(
            [b_qkv[2 * D + h * HD:2 * D + (h + 1) * HD] for h in heads])
        bvb = np.tile(bv[None, :], (P, 1)).astype(np.float32)      # [128, 256]

        wo = w_o[:, g * G * HD:(g + 1) * G * HD]                   # [D, 256]
        woT = np.ascontiguousarray(wo.T).astype(NPBF16)            # [256, D]

        in_maps.append({
            "xT": xT, "wqkT": wqkT, "bqk": bqk, "wvT": wvT, "bvb": bvb,
            "cosq": cosq, "sinq": sinq, "cosk": cosk, "sink": sink,
            "woT": woT,
        })
    return in_maps


def _run(inputs, trace=False):
    nc = _get_nc()
    in_maps = _shard_inputs(**inputs)
    res = run_bass_kernel_spmd(
        nc, in_maps, core_ids=list(range(NCORES)), trace=trace)
    b_o = np.asarray(inputs["b_o"], np.float32)
    out = np.zeros((B, S, D), np.float32)
    for c in range(NCORES):
        out[c // 4] += res.results[c]["out"].astype(np.float32)
    out += b_o[None, None, :]
    return out, res


def kernel(**inputs):
    out, _ = _run(inputs, trace=False)
    return out
